# revision 6
# baseline (speedup 1.0000x reference)
"""PixelRNN row-LSTM layer on 8 Trainium2 NeuronCores.

Strategy: data-parallel over batch (B=8 -> 1 image per core). Each core runs
the full H=128 row recurrence for its image:
  gates(r) = W_is_masked * X(r)  (1x3 masked conv, taps dx in {-1,0})
           + W_ss * h(r-1)       (1x3 conv, taps dx in {-1,0,+1})
  f,i,o = sigmoid(...), g = tanh(...), c = f*c + i*g, h = o*tanh(c)

Layout per core:
  - All matmuls in bf16 (PSUM accumulates fp32); c kept fp32; gates/h bf16.
  - Gate channels permuted host-side to gate-major order so each gate is one
    contiguous [96,128] block; each gate chunk gets its own PSUM bank
    (4 banks x 2 row-phases = all 8 banks).
  - Conv taps realized as column-shifted sub-range matmuls accumulating into
    the same PSUM bank (no padding needed; boundary columns simply don't
    receive the out-of-range tap, which is the correct zero contribution).
  - lhsT weight tiles are [K=96(c_in), M=128(gate pad)] so FWL stays enabled.
"""

import sys
import numpy as np

sys.path.insert(0, "/opt/trn_rl_repo")

import ml_dtypes

BF16 = ml_dtypes.bfloat16

B, C, H, W = 8, 96, 128, 128
O = 96
G4 = 4 * O  # 384
NCORES = 8
# gate chunk order in emission: i, g, f, o  (gate type index in reference: f=0,i=1,o=2,g=3)
CHUNK_GATE = [1, 3, 0, 2]  # chunk k holds reference-gate CHUNK_GATE[k]
SIG_CHUNKS = [0, 2, 3]  # chunks needing sigmoid (i, f, o)
TANH_CHUNK = 1  # g

_prog_cache = {}


def _make_mask_center(c_out, c_in):
    # center-tap mask (mask type 'B'): R out sees R in; G sees R,G; B sees all
    m = np.zeros((c_out, c_in), dtype=np.float32)
    co, ci = c_out // 3, c_in // 3
    m[:co, :ci] = 1.0
    m[co : 2 * co, : 2 * ci] = 1.0
    m[2 * co :, :] = 1.0
    return m


def _prep_weights(W_is, b_is, W_ss, b_ss):
    """Returns (w_pack [96, 20*128] bf16, bias_pack [96,4] f32).

    Tile idx layout: idx = q*4 + k for ss taps q in 0..2, chunks k in 0..3;
    idx = 12 + q*4 + k for is taps q in 0..1.
    w_pack[c, idx*128 + m] = weight[gate_channel(k,m), c, tap q], m<96.
    """
    Wm = W_is[:, :, 0, :].astype(np.float32).copy()  # (384, 96, 3)
    Wm[:, :, 2] = 0.0  # right tap masked
    Wm[:, :, 1] *= _make_mask_center(G4, C)  # center tap block mask
    # gate channel map: chunk k, m=color*32+j -> reference channel color*128 + gate*32 + j
    gmap = np.zeros((4, O), dtype=np.int64)
    for k, gt in enumerate(CHUNK_GATE):
        for color in range(3):
            for j in range(32):
                gmap[k, color * 32 + j] = color * 128 + gt * 32 + j

    w_pack = np.zeros((C, 20 * 128), dtype=np.float32)
    for q in range(3):
        for k in range(4):
            idx = q * 4 + k
            # W_ss[g, c, q] -> [c, m]
            w_pack[:, idx * 128 : idx * 128 + O] = W_ss[gmap[k], :, q].T
    for q in range(2):
        for k in range(4):
            idx = 12 + q * 4 + k
            w_pack[:, idx * 128 : idx * 128 + O] = Wm[gmap[k], :, q].T
    bias_pack = np.zeros((C, 4), dtype=np.float32)
    for k in range(4):
        bias_pack[:O, k] = (
            b_is.astype(np.float32)[gmap[k]] + b_ss.astype(np.float32)[gmap[k]]
        )
    return w_pack.astype(BF16), bias_pack


def _build_program():
    import concourse.bass as bass
    import concourse.bacc as bacc
    import concourse.tile as tile
    from concourse import mybir

    dt = mybir.dt
    AF = mybir.ActivationFunctionType

    nc = bacc.Bacc("TRN2", target_bir_lowering=False, debug=False, num_devices=NCORES)

    x_dram = nc.dram_tensor("Xin", [C, H * W], dt.bfloat16, kind="ExternalInput")
    w_dram = nc.dram_tensor("Wpack", [C, 20 * 128], dt.bfloat16, kind="ExternalInput")
    b_dram = nc.dram_tensor("Bpack", [C, 4], dt.float32, kind="ExternalInput")
    out_dram = nc.dram_tensor("Out", [C, H * W], dt.bfloat16, kind="ExternalOutput")

    XCH = 16  # X rows per DMA chunk

    with tile.TileContext(nc) as tc:
        with (
            tc.tile_pool(name="consts", bufs=1) as consts,
            tc.tile_pool(name="xbuf", bufs=1) as xbuf,
            tc.tile_pool(name="state", bufs=2) as state,
            tc.tile_pool(name="gates", bufs=2) as gates,
            tc.tile_pool(name="psum", bufs=2, space="PSUM") as psum,
        ):
            wt = consts.tile([C, 20 * 128], dt.bfloat16)
            nc.sync.dma_start(wt[:], w_dram[:])
            bt = consts.tile([C, 4], dt.float32)
            nc.sync.dma_start(bt[:], b_dram[:])

            def w_ss(q, k):
                i = q * 4 + k
                return wt[:, i * 128 : (i + 1) * 128]

            def w_is(q, k):
                i = 12 + q * 4 + k
                return wt[:, i * 128 : (i + 1) * 128]

            xch = []
            for ci in range(H // XCH):
                xt = xbuf.tile([C, XCH * W], dt.bfloat16, tag=f"x{ci}", name=f"x{ci}")
                nc.sync.dma_start(xt[:], x_dram[:, ci * XCH * W : (ci + 1) * XCH * W])
                xch.append(xt)

            h_prev = state.tile([C, W], dt.bfloat16, tag="h", name="h_init")
            nc.vector.memset(h_prev[:], 0.0)
            c_prev = state.tile([C, W], dt.float32, tag="c", name="c_init")
            nc.vector.memset(c_prev[:], 0.0)

            for r in range(H):
                xrow = xch[r // XCH][:, (r % XCH) * W : (r % XCH + 1) * W]
                ps = []
                for k in range(4):
                    p = psum.tile([128, W], dt.float32, tag=f"ps{k}", name=f"ps{k}_{r}")
                    ps.append(p)
                    # x-part: taps q=1 (center, full) then q=0 (left, shifted)
                    nc.tensor.matmul(p[:, 0:W], w_is(1, k), xrow[:, 0:W], start=True, stop=False)
                    nc.tensor.matmul(p[:, 1:W], w_is(0, k), xrow[:, 0 : W - 1], start=False, stop=False)
                    # h-part: q=1 (center), q=0 (left), q=2 (right)
                    nc.tensor.matmul(p[:, 0:W], w_ss(1, k), h_prev[:, 0:W], start=False, stop=False)
                    nc.tensor.matmul(p[:, 1:W], w_ss(0, k), h_prev[:, 0 : W - 1], start=False, stop=False)
                    nc.tensor.matmul(p[:, 0 : W - 1], w_ss(2, k), h_prev[:, 1:W], start=False, stop=True)

                i_t = gates.tile([O, W], dt.bfloat16, tag="i", name=f"i_{r}")
                nc.scalar.activation(i_t[:], ps[0][0:O, :], AF.Sigmoid, bias=bt[:, 0:1])
                g_t = gates.tile([O, W], dt.bfloat16, tag="g", name=f"g_{r}")
                nc.scalar.activation(g_t[:], ps[1][0:O, :], AF.Tanh, bias=bt[:, 1:2])
                f_t = gates.tile([O, W], dt.bfloat16, tag="f", name=f"f_{r}")
                nc.scalar.activation(f_t[:], ps[2][0:O, :], AF.Sigmoid, bias=bt[:, 2:3])
                o_t = gates.tile([O, W], dt.bfloat16, tag="o", name=f"o_{r}")
                nc.scalar.activation(o_t[:], ps[3][0:O, :], AF.Sigmoid, bias=bt[:, 3:4])

                ig = gates.tile([O, W], dt.float32, tag="ig", name=f"ig_{r}")
                nc.vector.tensor_mul(ig[:], i_t[:], g_t[:])
                fc = gates.tile([O, W], dt.float32, tag="fc", name=f"fc_{r}")
                nc.vector.tensor_mul(fc[:], f_t[:], c_prev[:])
                c_new = state.tile([C, W], dt.float32, tag="c", name=f"c_{r}")
                nc.vector.tensor_add(c_new[:], ig[:], fc[:])
                tc_t = gates.tile([O, W], dt.bfloat16, tag="tc", name=f"tc_{r}")
                nc.scalar.activation(tc_t[:], c_new[:], AF.Tanh)
                h_new = state.tile([C, W], dt.bfloat16, tag="h", name=f"h_{r}")
                nc.vector.tensor_mul(h_new[:], o_t[:], tc_t[:])

                nc.sync.dma_start(out_dram[:, r * W : (r + 1) * W], h_new[:])

                h_prev, c_prev = h_new, c_new

    nc.compile()
    return nc


def _get_program():
    if "nc" not in _prog_cache:
        _prog_cache["nc"] = _build_program()
    return _prog_cache["nc"]


def _run(inputs, trace=False):
    from concourse.bass_utils import run_bass_kernel_spmd

    X = np.asarray(inputs["X"])
    w_pack, bias_pack = _prep_weights(
        np.asarray(inputs["W_is"]),
        np.asarray(inputs["b_is"]),
        np.asarray(inputs["W_ss"]),
        np.asarray(inputs["b_ss"]),
    )
    nc = _get_program()
    in_maps = []
    for b in range(NCORES):
        in_maps.append(
            {
                "Xin": X[b].reshape(C, H * W).astype(BF16),
                "Wpack": w_pack,
                "Bpack": bias_pack,
            }
        )
    res = run_bass_kernel_spmd(nc, in_maps, list(range(NCORES)), trace=trace)
    out = np.stack(
        [
            res.results[b]["Out"].astype(np.float32).reshape(C, H, W)
            for b in range(NCORES)
        ],
        axis=0,
    )
    return out, res.exec_time_ns


def kernel(**inputs):
    out, _ = _run(inputs, trace=False)
    return out
